# revision 27
# baseline (speedup 1.0000x reference)
"""AWLoss1D batched-Toeplitz-solve loss on 8 Trainium2 NeuronCores.

Math (per batch row b of 512):
  D_b = (511x256) Toeplitz of target_b;  A_b = D^T D + eps*I  (256x256 SPD
  symmetric Toeplitz);  v_b = A_b^{-1} (D^T pad(recon_b));
  loss = sum_b 0.5*||T.v||/||v||.

Device algorithm (64 systems per core, pure data parallel):
  * A_b embeds exactly in the 512-circulant whose eigenvalues are
    lam_b = |FFT_512(target_b zero-padded)|^2 (>= 0 by construction):
    A_b = P (C_b + eps I) P^T.  lam symmetric => C_b is diagonalized by the
    REAL 512-point Hartley transform H5, SHARED by all batches, so batched
    matvecs are plain PE matmuls with batch on the free dim.
  * Preconditioner: T.Chan optimal circulant of A_b (256-point Hartley H2),
    spectrum lam_chan = W @ lam with W precomputed on the host.
  * PCG with ALL state kept spectral: xh = H5 x, ph = H5 p (512-spectra),
    rh = H2 r (256-spectrum).  One iteration needs only two constant-matrix
    products, KM = (1/512) H2 H5^T (maps lamt.ph into the H2-domain residual
    update) and K2 = (1/256) H5 H2^T (lifts the preconditioned residual back
    to the H5 domain); dot products come from Parseval as ones-matmul
    partition reductions, already transposed to [1, batch] so the per-batch
    alpha/beta broadcast is a single rank-1 ones outer product.
  * RHS spectrum directly: bh = (H2 BC^T) Zre + (H2 BS^T) Zimn where
    Z = conj(FFT(target)).FFT(recon), with the pad-127 shift and 1/512
    folded into BC/BS on the host.
  * Finale: x = (1/512) H5^T xh (one matmul set), PE transpose to
    [batch, 256], ratio reduction, ones-matmul partition sum -> scalar;
    host sums the 8 per-core partials.
"""
import functools

import numpy as np

B, HH, N, NCORES = 512, 256, 512, 8
BPC = B // NCORES  # 64 batches per core
EPS = 1e-4
FLOOR = 0.1
ITERS = 9


@functools.lru_cache(maxsize=1)
def _host_consts():
    """Constant matrices, computed in f64, stored f32, laid out for the PE
    (lhsT convention: [contraction, out])."""
    n5 = np.arange(N)
    n2 = np.arange(HH)
    ang5 = 2.0 * np.pi * np.outer(n5, n5) / N
    cas5 = np.cos(ang5) + np.sin(ang5)
    ang2 = 2.0 * np.pi * np.outer(n2, n2) / HH
    cas2 = np.cos(ang2) + np.sin(ang2)
    H5 = cas5[:, :HH]                                   # [512f, 256n]
    H2 = cas2                                           # [256g, 256n]

    KMT = ((H2 @ H5.T) / N).T.copy()                    # lhsT [512f, 256g]
    K2T = ((H5 @ H2.T) / HH).T.copy()                   # lhsT [256g, 512f]
    IH5 = (cas5 / N)[:, :HH].copy()                     # lhsT [512f, 256n]
    FCT = np.cos(ang5)[:, :HH].T.copy()                 # lhsT [256n, 512f]
    FST = (-np.sin(ang5))[:, :HH].T.copy()              # lhsT [256n, 512f]
    angb = 2.0 * np.pi * np.outer(n5, n2 - 127.0) / N   # [f, j]
    BCm = np.cos(angb) / N                              # [512 f, 256 j]
    BSm = np.sin(angb) / N
    BCHT = (H2 @ BCm.T).T.copy()                        # lhsT [512f, 256g]
    BSHT = (H2 @ BSm.T).T.copy()                        # lhsT [512f, 256g]
    # lam_chan = W @ lam ; W = DCT256 @ CW @ RHO
    RHO = np.cos(2.0 * np.pi * np.outer(n2, n5) / N) / N
    CW = np.zeros((HH, HH))
    CW[n2, n2] += (HH - n2) / HH
    CW[n2, (HH - n2) % HH] += n2 / HH
    DCT = np.cos(2.0 * np.pi * np.outer(n2, n2) / HH)
    W = DCT @ CW @ RHO                                  # [256 g, 512 f]
    WT = W.T.copy()                                     # lhsT [512 f, 256 g]
    # device uses lamt = lam + EPS; cv makes the mu denominator lam_chan+EPS
    cv = (EPS * (1.0 - W.sum(axis=1)))[:, None].copy()  # [256 g, 1]

    x = np.linspace(-10.0, 10.0, HH)
    dx = (x[-1] - x[0]) / (HH - 1)
    dispx = (HH % 2 - 1) / 2.0
    g = -np.exp(-((x - dx * dispx) ** 2) / 2.0)
    g = g + np.max(np.abs(g))
    Tw = g / np.max(np.abs(g))
    TB = np.broadcast_to(Tw[None, :], (BPC, HH)).copy()

    f32 = lambda a: np.ascontiguousarray(a, dtype=np.float32)
    return {
        "KMT": f32(KMT), "K2T": f32(K2T), "IH5": f32(IH5),
        "FCT": f32(FCT), "FST": f32(FST), "BCHT": f32(BCHT), "BSHT": f32(BSHT),
        "WT": f32(WT), "CV": f32(cv), "TB": f32(TB),
    }


@functools.lru_cache(maxsize=1)
def _program():
    import concourse.bacc as bacc
    import concourse.mybir as mybir
    import concourse.tile as tile
    from concourse.masks import make_identity

    F32 = mybir.dt.float32
    AL = mybir.AluOpType

    nc = bacc.Bacc(target_bir_lowering=False)

    d_tT = nc.dram_tensor("tT", [HH, BPC], F32, kind="ExternalInput")
    d_rT = nc.dram_tensor("rT", [HH, BPC], F32, kind="ExternalInput")
    dm = {}
    for name, shp in [
        ("KMT", [N, HH]), ("K2T", [HH, N]), ("IH5", [N, HH]),
        ("FCT", [HH, N]), ("FST", [HH, N]), ("BCHT", [N, HH]),
        ("BSHT", [N, HH]), ("WT", [N, HH]), ("CV", [HH, 1]), ("TB", [BPC, HH]),
    ]:
        dm[name] = nc.dram_tensor(name, shp, F32, kind="ExternalInput")
    d_out = nc.dram_tensor("out", [1, 1], F32, kind="ExternalOutput")

    with tile.TileContext(nc) as tc:
        with (
            tc.tile_pool(name="consts", bufs=1) as consts,
            tc.tile_pool(name="state", bufs=1) as state,
            tc.tile_pool(name="work", bufs=2) as work,
            tc.tile_pool(name="psum", bufs=1, space="PSUM") as psum,
        ):
            def loadc(name, eng=None):
                chunks = dm[name].shape[0] // 128
                t = consts.tile([128, chunks, dm[name].shape[1]], F32, tag=name)
                (eng or nc.sync).dma_start(
                    out=t, in_=dm[name].ap().rearrange("(c p) x -> p c x", p=128)
                )
                return t

            # target and recon transposed, concatenated on the free dim so the
            # FFT matmuls of both run as one pass with free dim 128
            tr = state.tile([128, 2, 2, BPC], F32, tag="tr")
            nc.sync.dma_start(
                out=tr[:, :, 0, :], in_=d_tT.ap().rearrange("(c p) x -> p c x", p=128))
            nc.sync.dma_start(
                out=tr[:, :, 1, :], in_=d_rT.ap().rearrange("(c p) x -> p c x", p=128))

            # spread constant loads across the three DMA-capable queues
            # (sync, scalar=Activation HWDGE; gpsimd SWDGE) so they overlap
            fct, fst = loadc("FCT"), loadc("FST", nc.scalar)
            wt, cvt = loadc("WT", nc.gpsimd), loadc("CV", nc.gpsimd)
            bcht, bsht = loadc("BCHT"), loadc("BSHT", nc.scalar)
            kmt, k2t = loadc("KMT"), loadc("K2T", nc.scalar)
            ih5 = loadc("IH5", nc.gpsimd)
            tb = consts.tile([BPC, HH], F32, tag="TB")
            nc.scalar.dma_start(out=tb, in_=dm["TB"].ap())

            ident = consts.tile([128, 128], F32, tag="ident")
            make_identity(nc, ident)
            ones = consts.tile([128, 1], F32, tag="ones")
            nc.vector.memset(ones, 1.0)
            twos = consts.tile([128, 1], F32, tag="twos")
            nc.vector.memset(twos, 2.0)
            onesr = consts.tile([1, 128], F32, tag="onesr")
            nc.vector.memset(onesr, 1.0)
            # preload the ACT Sqrt table off the critical path (the finale's
            # ratio sqrt would otherwise pay the ~1.3us table load serially)
            sqwarm = consts.tile([1, 1], F32, tag="sqwarm")
            nc.scalar.activation(
                out=sqwarm, in_=ones[0:1, :],
                func=mybir.ActivationFunctionType.Sqrt, scale=1.0)

            # ---- generic [K-chunks x out-tiles] matmul into a packed PSUM ----
            def mms(lhsT, src, kchunks, otiles, ptag):
                ps = psum.tile([128, otiles, BPC], F32, tag=ptag)
                for ot in range(otiles):
                    for kc in range(kchunks):
                        nc.tensor.matmul(
                            ps[:, ot, :],
                            lhsT[:, kc, ot * 128:(ot + 1) * 128],
                            src[:, kc, :],
                            start=(kc == 0), stop=(kc == kchunks - 1),
                        )
                return ps

            def rdot(a_sb, b_sb, lhs_col, dtag, ptag, eng):
                """[1, BPC] PSUM = sum over partition-chunks of lhs_col.(a.b)"""
                kchunks = a_sb.shape[1]
                dt = work.tile([128, kchunks, BPC], F32, tag=dtag)
                eng.tensor_mul(dt, a_sb, b_sb)
                ps = psum.tile([1, BPC], F32, tag=ptag)
                for kc in range(kchunks):
                    nc.tensor.matmul(
                        ps, lhs_col, dt[:, kc, :],
                        start=(kc == 0), stop=(kc == kchunks - 1),
                    )
                return ps

            def bcast_row(row_sb, tag):
                """[1, BPC] SBUF -> [128, BPC] SBUF partition broadcast"""
                ob = work.tile([128, BPC], F32, tag=tag)
                nc.gpsimd.partition_broadcast(ob, row_sb)
                return ob

            def bx(ps_2d, chunks):
                return ps_2d[:, None, :].broadcast_to([128, chunks, BPC])

            # ---- setup: [u | R] = FFT of [t_pad | r_pad]; lamt = |u|^2+eps ----
            re_ps = psum.tile([128, 4, 2, BPC], F32, tag="pa")
            im_ps = psum.tile([128, 4, 2, BPC], F32, tag="pb")
            for ot in range(4):
                for kc in range(2):
                    src = tr[:, kc, :, :].rearrange("p a b -> p (a b)")
                    nc.tensor.matmul(
                        re_ps[:, ot, :, :].rearrange("p a b -> p (a b)"),
                        fct[:, kc, ot * 128:(ot + 1) * 128], src,
                        start=(kc == 0), stop=(kc == 1))
                    nc.tensor.matmul(
                        im_ps[:, ot, :, :].rearrange("p a b -> p (a b)"),
                        fst[:, kc, ot * 128:(ot + 1) * 128], src,
                        start=(kc == 0), stop=(kc == 1))
            ure_ps, rre_ps = re_ps[:, :, 0, :], re_ps[:, :, 1, :]
            uim_ps, rim_ps = im_ps[:, :, 0, :], im_ps[:, :, 1, :]
            ure = state.tile([128, 4, BPC], F32, tag="ure")
            uim = state.tile([128, 4, BPC], F32, tag="uim")
            nc.scalar.copy(ure, ure_ps)
            nc.scalar.copy(uim, uim_ps)
            lamt = state.tile([128, 4, BPC], F32, tag="lamt")
            sqre = work.tile([128, 4, BPC], F32, tag="sqre")
            sq2 = work.tile([128, 4, BPC], F32, tag="sq2")
            nc.scalar.square(sqre, ure_ps)
            nc.scalar.square(sq2, uim_ps)
            nc.vector.tensor_add(lamt, sqre, sq2)
            nc.vector.tensor_scalar_add(lamt, lamt, EPS)

            # ---- mu = 1/max(W@lamt + cv, FLOOR) ----
            lc_ps = mms(wt, lamt, 4, 2, "pc")
            mu = state.tile([128, 2, BPC], F32, tag="mu")
            for gt in range(2):
                nc.vector.tensor_scalar(
                    out=mu[:, gt, :], in0=lc_ps[:, gt, :],
                    scalar1=cvt[:, gt, :], scalar2=FLOOR,
                    op0=AL.add, op1=AL.max)
            nc.vector.reciprocal(mu, mu)

            # ---- Z = conj(u).R; bh = BCH@Zre + BSH@Zimn ----
            t1 = work.tile([128, 4, BPC], F32, tag="t1")
            t2 = work.tile([128, 4, BPC], F32, tag="t2")
            zre = work.tile([128, 4, BPC], F32, tag="zre")
            zimn = work.tile([128, 4, BPC], F32, tag="zimn")
            nc.vector.tensor_mul(t1, ure, rre_ps)
            nc.vector.tensor_mul(t2, uim, rim_ps)
            nc.vector.tensor_add(zre, t1, t2)
            nc.vector.tensor_mul(t1, uim, rre_ps)
            nc.vector.tensor_mul(t2, ure, rim_ps)
            nc.vector.tensor_sub(zimn, t1, t2)

            bh_ps = psum.tile([128, 2, BPC], F32, tag="pc")
            for gt in range(2):
                for ft in range(4):
                    nc.tensor.matmul(
                        bh_ps[:, gt, :], bcht[:, ft, gt * 128:(gt + 1) * 128],
                        zre[:, ft, :], start=(ft == 0), stop=False)
                for ft in range(4):
                    nc.tensor.matmul(
                        bh_ps[:, gt, :], bsht[:, ft, gt * 128:(gt + 1) * 128],
                        zimn[:, ft, :], start=False, stop=(ft == 3))
            bh = state.tile([128, 2, BPC], F32, tag="bh")
            nc.scalar.copy(bh, bh_ps)

            # ---- x0h = K2(mu.bh); r0h = bh - KM(lamt.x0h); p0h = K2(mu.r0h) ----
            xh = state.tile([128, 4, BPC], F32, tag="xh")
            ph = state.tile([128, 4, BPC], F32, tag="ph")
            rh = state.tile([128, 2, BPC], F32, tag="rh")
            rz = state.tile([1, BPC], F32, tag="rz")
            irz = state.tile([1, BPC], F32, tag="irz")

            sh = work.tile([128, 2, BPC], F32, tag="sh")
            nc.vector.tensor_mul(sh, mu, bh_ps)
            x0_ps = mms(k2t, sh, 2, 4, "pa")
            th = work.tile([128, 4, BPC], F32, tag="th")
            nc.vector.tensor_mul(th, lamt, x0_ps)
            nc.scalar.copy(xh, x0_ps)
            g0_ps = mms(kmt, th, 4, 2, "pb")
            nc.vector.tensor_sub(rh, bh, g0_ps)
            nc.vector.tensor_mul(sh, mu, rh)
            p0_ps = mms(k2t, sh, 2, 4, "pa")
            nc.scalar.copy(ph, p0_ps)
            rz_ps = rdot(sh, rh, twos, "dt2", "sd", nc.vector)
            nc.vector.tensor_copy(rz, rz_ps)
            nc.vector.reciprocal(irz, rz)

            # ---- PCG iterations, all spectral ----
            for _ in range(ITERS):
                th = work.tile([128, 4, BPC], F32, tag="th")
                nc.vector.tensor_mul(th, lamt, ph)
                pq_ps = rdot(th, ph, ones, "dt", "sc", nc.vector)
                gh_ps = mms(kmt, th, 4, 2, "pb")

                ipq = work.tile([1, BPC], F32, tag="ipq")
                nc.vector.reciprocal(ipq, pq_ps)
                al = work.tile([1, BPC], F32, tag="al")
                nc.vector.tensor_mul(al, rz, ipq)
                alB = bcast_row(al, "alB")

                # x-hat += alpha p-hat   (gpsimd: off critical path, SBUF only)
                tt4 = work.tile([128, 4, BPC], F32, tag="tt4")
                nc.gpsimd.tensor_mul(tt4, ph, bx(alB, 4))
                nc.gpsimd.tensor_add(xh, xh, tt4)

                # r-hat -= alpha g-hat
                tt2 = work.tile([128, 2, BPC], F32, tag="tt2")
                nc.vector.tensor_mul(tt2, bx(alB, 2), gh_ps)
                nc.vector.tensor_sub(rh, rh, tt2)

                sh = work.tile([128, 2, BPC], F32, tag="sh")
                nc.vector.tensor_mul(sh, mu, rh)
                # beta-dot issued before K2 so its PE matmuls run first and the
                # beta scalar chain overlaps K2's eight matmuls
                rz2_ps = rdot(sh, rh, twos, "dt2", "sd", nc.vector)
                k2_ps = mms(k2t, sh, 2, 4, "pa")

                be = work.tile([1, BPC], F32, tag="be")
                nc.vector.tensor_mul(be, irz, rz2_ps)
                beB = bcast_row(be, "beB")

                t3 = work.tile([128, 4, BPC], F32, tag="t3")
                nc.vector.tensor_mul(t3, ph, bx(beB, 4))
                nc.vector.tensor_add(ph, k2_ps, t3)
                # bookkeeping for the next iteration, off the critical path
                nc.vector.tensor_copy(rz, rz2_ps)
                nc.vector.reciprocal(irz, rz)

            # ---- finale ----
            x_ps = mms(ih5, xh, 4, 2, "pc")
            x_sb = work.tile([128, 2, BPC], F32, tag="xsb")
            nc.scalar.copy(x_sb, x_ps)
            v64 = work.tile([BPC, HH], F32, tag="v64")
            for nt in range(2):
                vt_ps = psum.tile([BPC, 128], F32, tag="sa")
                nc.tensor.transpose(vt_ps, x_sb[:, nt, :], ident)
                nc.scalar.copy(v64[:, nt * 128:(nt + 1) * 128], vt_ps)
            tv = work.tile([BPC, HH], F32, tag="tv")
            nc.vector.tensor_mul(tv, v64, tb)
            num2 = work.tile([BPC, 1], F32, tag="num2")
            den2 = work.tile([BPC, 1], F32, tag="den2")
            sq = work.tile([BPC, HH], F32, tag="sq")
            nc.vector.tensor_mul(sq, tv, tv)
            nc.vector.tensor_reduce(num2, sq, mybir.AxisListType.X, AL.add)
            nc.vector.tensor_mul(sq, v64, v64)
            nc.vector.tensor_reduce(den2, sq, mybir.AxisListType.X, AL.add)
            nc.vector.reciprocal(den2, den2)
            rat = work.tile([BPC, 1], F32, tag="rat")
            nc.vector.tensor_mul(rat, num2, den2)
            nc.scalar.activation(
                out=rat, in_=rat, func=mybir.ActivationFunctionType.Sqrt,
                scale=0.25)
            ls_ps = psum.tile([1, 1], F32, tag="sb")
            nc.tensor.matmul(ls_ps, rat, ones[:BPC, :], start=True, stop=True)
            out_sb = work.tile([1, 1], F32, tag="outsb")
            nc.vector.tensor_copy(out_sb, ls_ps)
            nc.sync.dma_start(out=d_out.ap(), in_=out_sb)

    nc.finalize()
    return nc


def kernel(recon: np.ndarray, target: np.ndarray) -> np.ndarray:
    from concourse.bass_utils import run_bass_kernel_spmd

    consts = _host_consts()
    nc = _program()

    in_maps = []
    for c in range(NCORES):
        sl = slice(c * BPC, (c + 1) * BPC)
        m = dict(consts)
        m["tT"] = np.ascontiguousarray(target[sl].T, dtype=np.float32)
        m["rT"] = np.ascontiguousarray(recon[sl].T, dtype=np.float32)
        in_maps.append(m)

    res = run_bass_kernel_spmd(nc, in_maps, core_ids=list(range(NCORES)))
    kernel._last_results = res  # for test.py introspection (profiling)
    total = np.float32(0.0)
    for c in range(NCORES):
        total += res.results[c]["out"][0, 0]
    return np.float32(total)
